# revision 1
# baseline (speedup 1.0000x reference)
"""Trainium2 Bass kernel for nn_CCGGenerator (LSTM encoder + attention decoder).

Sharding: data-parallel, batch 128 -> 16 per core across 8 cores.
All weights replicated. Self-contained; everything hardcoded.

Per-core design (B=16):
- Encoder gates computed transposed: gates.T [2048, 16] as 16 PSUM tiles
  [128, 16] packed in one [128, 256] region (col = m*16 + b). lhsT = Whh.T
  tiles [128,128] bf16 (weight-stationary), rhs = h.T [128, 16] bf16.
- The whole condition-latent CL lives in SBUF: cl_sb [128, N*64] bf16 with
  col = n*64 + k*16 + b  (k = hidden 128-chunk). The h-update writes its
  [128, 64] slice directly; next step's matmuls read it back. No DMA in the
  recurrence at all.
- xg = x @ Wih.T + (bih+bhh) precomputed per 8-step chunk into SBUF fp32
  (bias via augmented ones-row).
- Decoder: 24 steps into hd_sb [128, 4*24*16] (col = k*384 + t*16 + b).
- Attention per b: scores [24, N] from strided cl_sb reads (CL.T is native);
  softmax on free dim; CL_b n-partitioned tiles via PE transposes for ctx.
"""
import sys
sys.path.insert(0, "/opt/trn_rl_repo")

import numpy as np
import ml_dtypes
from contextlib import ExitStack

import concourse.bass as bass
import concourse.tile as tile
from concourse import bacc, mybir
from concourse.bass_utils import run_bass_kernel_spmd

F32 = mybir.dt.float32
BF16 = mybir.dt.bfloat16
AF = mybir.ActivationFunctionType
OP = mybir.AluOpType
BF = ml_dtypes.bfloat16

NCORES = 8
B = 16          # batch per core
N_STEPS = 1024  # encoder sequence length
SCH = 8         # steps per xg chunk
C = 32
H = 512
G = 2048        # 4H
T = 24
KH = 4          # hidden 128-chunks
M16 = 16        # gate-dim 128-chunks


def build_program(n_steps=N_STEPS):
    nch = n_steps // SCH
    nac = n_steps // 128  # attention n-chunks
    nc = bacc.Bacc("TRN2", target_bir_lowering=False, debug=False,
                   num_devices=NCORES)

    p_cond = nc.declare_dram_parameter("cond_aT", [C + 1, n_steps * B], BF16, isOutput=False)
    p_wih = nc.declare_dram_parameter("wihT_a", [C + 1, G], BF16, isOutput=False)
    p_whh = nc.declare_dram_parameter("whhT", [128, 64 * 128], BF16, isOutput=False)
    p_wcell = nc.declare_dram_parameter("wcellT", [128, 64 * 128], BF16, isOutput=False)
    p_bcell = nc.declare_dram_parameter("bcell_bc", [128, 256], F32, isOutput=False)
    p_wout = nc.declare_dram_parameter("woutT", [128, 8 * C], BF16, isOutput=False)
    p_bout = nc.declare_dram_parameter("bout_bc", [T, C], F32, isOutput=False)
    p_id = nc.declare_dram_parameter("ident", [128, 128], BF16, isOutput=False)
    p_out = nc.declare_dram_parameter("out", [B, T, C], F32, isOutput=True)

    with tile.TileContext(nc) as tc, ExitStack() as ctx:
        const = ctx.enter_context(tc.tile_pool(name="const", bufs=1))

        cl_sb = const.tile([128, n_steps * KH * B], BF16, tag="cl_sb")  # 16 MB
        hd_sb = const.tile([128, KH * T * B], BF16, tag="hd_sb")
        wout_sb = const.tile([128, 8 * C], BF16, tag="wout_sb")
        nc.sync.dma_start(wout_sb[:], p_wout[:])
        bout_sb = const.tile([T, C], F32, tag="bout_sb")
        nc.sync.dma_start(bout_sb[:], p_bout[:])
        id_sb = const.tile([128, 128], BF16, tag="id_sb")
        nc.sync.dma_start(id_sb[:], p_id[:])
        bcell_sb = const.tile([128, 256], F32, tag="bcell_sb")
        nc.sync.dma_start(bcell_sb[:], p_bcell[:])

        cl3 = cl_sb[:].rearrange("p (n k b) -> p n k b", k=KH, b=B)
        hd_v = hd_sb[:].rearrange("p (k t b) -> p k t b", k=KH, t=T)

        def lstm_tail(gps, xg_like, bias_sb, st_pool, gtmp_pool, c_f, h_out):
            """gps [128,256] PSUM -> gates -> state update; h written to h_out
            ([128, (4,16)] AP, bf16). c_f None => decoder variant (c = i*g)."""
            ga = gtmp_pool.tile([128, 256], F32, tag="ga")
            if xg_like is not None:
                # per-m adds keep every AP flat 2-level for walrus encoding
                for m in range(M16):
                    nc.vector.tensor_tensor(ga[:, bass.ts(m, B)], gps[:, bass.ts(m, B)],
                                            xg_like(m), op=OP.add)
            else:
                nc.vector.tensor_tensor(ga[:], gps[:], bias_sb[:], op=OP.add)
            nc.scalar.activation(ga[:, 0:128], ga[:, 0:128], AF.Sigmoid)     # i, f
            nc.scalar.activation(ga[:, 128:192], ga[:, 128:192], AF.Tanh)    # g
            nc.scalar.activation(ga[:, 192:256], ga[:, 192:256], AF.Sigmoid)  # o
            if c_f is not None:
                ig = st_pool.tile([128, 64], F32, tag="ig")
                nc.vector.tensor_tensor(ig[:], ga[:, 0:64], ga[:, 128:192], op=OP.mult)
                c2 = st_pool.tile([128, 64], F32, tag="c2")
                nc.vector.tensor_tensor(c2[:], ga[:, 64:128], c_f[:], op=OP.mult)
                nc.vector.tensor_tensor(c_f[:], c2[:], ig[:], op=OP.add)
                csrc = c_f[:]
            else:
                cd = st_pool.tile([128, 64], F32, tag="cd")
                nc.vector.tensor_tensor(cd[:], ga[:, 0:64], ga[:, 128:192], op=OP.mult)
                csrc = cd[:]
            th = st_pool.tile([128, 64], F32, tag="th")
            nc.scalar.activation(th[:], csrc, AF.Tanh)
            for k in range(KH):
                nc.vector.tensor_tensor(h_out(k), ga[:, 192 + k * B:192 + (k + 1) * B],
                                        th[:, bass.ts(k, B)], op=OP.mult)

        # ---------------- encoder + decoder (shared gate/state pools) ----------------
        rec_pools = ExitStack()
        g_ps_pool = rec_pools.enter_context(tc.tile_pool(name="g_ps", bufs=2, space="PSUM"))
        gtmp_pool = rec_pools.enter_context(tc.tile_pool(name="gtmp", bufs=2))
        st_pool = rec_pools.enter_context(tc.tile_pool(name="st", bufs=2))
        with tc.tile_pool(name="enc", bufs=1) as enc_pool, \
             tc.tile_pool(name="xg", bufs=2) as xg_pool, \
             tc.tile_pool(name="xg_ps", bufs=2, space="PSUM") as xg_ps_pool:
            whh_sb = enc_pool.tile([128, 64 * 128], BF16, tag="whh_sb")
            nc.sync.dma_start(whh_sb[:], p_whh[:])
            wih_sb = enc_pool.tile([C + 1, G], BF16, tag="wih_sb")
            nc.sync.dma_start(wih_sb[:], p_wih[:])
            c_f = enc_pool.tile([128, KH * B], F32, tag="c_f")
            h0 = enc_pool.tile([128, KH * B], BF16, tag="h0")
            nc.any.memset(c_f[:], 0.0)
            nc.any.memset(h0[:], 0.0)

            for chv in range(nch):
                cond_ch = xg_pool.tile([C + 1, SCH * B], BF16, tag="cond_ch")
                nc.sync.dma_start(cond_ch[:], p_cond[:, bass.ts(chv, SCH * B)])
                xg_sb = xg_pool.tile([128, M16 * SCH * B], F32, tag="xg_sb")
                for m in range(M16):
                    xps = xg_ps_pool.tile([128, SCH * B], F32, tag="xps")
                    nc.tensor.matmul(xps[:], wih_sb[:, bass.ts(m, 128)],
                                     cond_ch[:], start=True, stop=True)
                    nc.scalar.copy(xg_sb[:, bass.ts(m, SCH * B)], xps[:])
                for s in range(SCH):
                    n = chv * SCH + s
                    hin = (lambda k: h0[:, bass.ts(k, B)]) if n == 0 else \
                        (lambda k, _n=n: cl_sb[:, (_n - 1) * 64 + k * B:(_n - 1) * 64 + (k + 1) * B])
                    gps = g_ps_pool.tile([128, 256], F32, tag="gps")
                    for m in range(M16):
                        for k in range(KH):
                            nc.tensor.matmul(gps[:, bass.ts(m, B)],
                                             whh_sb[:, bass.ts(m * KH + k, 128)],
                                             hin(k), start=(k == 0), stop=(k == KH - 1))
                    lstm_tail(gps,
                              lambda m, _s=s, _x=xg_sb: _x[:, m * SCH * B + _s * B:m * SCH * B + (_s + 1) * B],
                              None, st_pool, gtmp_pool, c_f,
                              lambda k, _n=n: cl_sb[:, _n * 64 + k * B:_n * 64 + (k + 1) * B])

        # ---------------- decoder ----------------
        with tc.tile_pool(name="dec", bufs=1) as dec_pool:
            wcell_sb = dec_pool.tile([128, 64 * 128], BF16, tag="wcell_sb")
            nc.sync.dma_start(wcell_sb[:], p_wcell[:])
            for t in range(T):
                if t == 0:
                    hin = lambda k: cl_sb[:, (n_steps - 1) * 64 + k * B:(n_steps - 1) * 64 + (k + 1) * B]
                else:
                    hin = lambda k, _t=t: hd_sb[:, k * T * B + (_t - 1) * B:k * T * B + _t * B]
                gps = g_ps_pool.tile([128, 256], F32, tag="gps")
                for m in range(M16):
                    for k in range(KH):
                        nc.tensor.matmul(gps[:, bass.ts(m, B)],
                                         wcell_sb[:, bass.ts(m * KH + k, 128)],
                                         hin(k), start=(k == 0), stop=(k == KH - 1))
                lstm_tail(gps, None, bcell_sb, st_pool, gtmp_pool, None,
                          lambda k, _t=t: hd_sb[:, k * T * B + _t * B:k * T * B + (_t + 1) * B])
        rec_pools.close()

        # ---------------- attention + output, per batch ----------------
        with tc.tile_pool(name="att_fix", bufs=2) as att_fix, \
             tc.tile_pool(name="scr_ps", bufs=1, space="PSUM") as scr_ps_pool, \
             tc.tile_pool(name="tp_ps", bufs=2, space="PSUM") as tp_ps_pool, \
             tc.tile_pool(name="ctx_ps", bufs=2, space="PSUM") as ctx_ps_pool:
            for b in range(B):
                # scores [24, n]: lhsT = hd strided, rhs = cl_sb strided (CL.T native)
                scr = scr_ps_pool.tile([T, n_steps], F32, tag="scr")
                scn = min(512, n_steps)
                for k in range(KH):
                    for j in range(n_steps // scn):
                        rhs = cl3[:, j * scn:(j + 1) * scn, k, b]
                        nc.tensor.matmul(scr[:, bass.ts(j, scn)], hd_v[:, k, :, b],
                                         rhs, start=(k == 0), stop=(k == KH - 1))
                nmx = att_fix.tile([T, 1], F32, tag="nmx")
                nc.vector.reduce_max(nmx[:], scr[:], axis=mybir.AxisListType.X, negate=True)
                ex = att_fix.tile([T, n_steps], F32, tag="ex")
                sm = att_fix.tile([T, 1], F32, tag="sm")
                nc.scalar.activation(ex[:], scr[:], AF.Exp, bias=nmx[:], accum_out=sm[:])
                rc = att_fix.tile([T, 1], F32, tag="rc")
                nc.vector.reciprocal(rc[:], sm[:])
                cof = att_fix.tile([T, n_steps], BF16, tag="cof")
                nc.vector.tensor_scalar(cof[:], ex[:], rc[:], None, op0=OP.mult)
                # coeff.T [n, 24] via PE transposes
                cT = att_fix.tile([128, nac * T], BF16, tag="cT")
                for j in range(nac):
                    tp = tp_ps_pool.tile([128, 128], BF16, tag="tp")
                    nc.tensor.transpose(tp[:, 0:T], cof[:, bass.ts(j, 128)], id_sb[0:T, 0:T])
                    nc.scalar.copy(cT[:, bass.ts(j, T)], tp[:, 0:T])
                # CL_b n-partitioned tiles via PE transposes
                clb = att_fix.tile([128, nac * KH * 128], BF16, tag="clb")
                for j in range(nac):
                    for k in range(KH):
                        tpc = tp_ps_pool.tile([128, 128], BF16, tag="tp")
                        nc.tensor.transpose(tpc[:], cl3[:, j * 128:(j + 1) * 128, k, b],
                                            id_sb[:, :])
                        nc.scalar.copy(clb[:, bass.ts(j * KH + k, 128)], tpc[:])
                # ctx.T [512, 24]
                ctxp = ctx_ps_pool.tile([128, KH * T], F32, tag="ctxp")
                for k in range(KH):
                    for j in range(nac):
                        nc.tensor.matmul(ctxp[:, bass.ts(k, T)],
                                         clb[:, bass.ts(j * KH + k, 128)],
                                         cT[:, bass.ts(j, T)],
                                         start=(j == 0), stop=(j == nac - 1))
                # out [24, 32]
                ob_ps = scr_ps_pool.tile([T, C], F32, tag="ob_ps")
                for jj in range(8):
                    lr = att_fix.tile([128, T], BF16, tag="lr")
                    src = hd_v[:, jj, :, b] if jj < KH else ctxp[:, bass.ts(jj - KH, T)]
                    nc.scalar.activation(lr[:], src, AF.Lrelu, alpha=0.01)
                    nc.tensor.matmul(ob_ps[:], lr[:], wout_sb[:, bass.ts(jj, C)],
                                     start=(jj == 0), stop=(jj == 7))
                ob = att_fix.tile([T, C], F32, tag="ob")
                nc.vector.tensor_tensor(ob[:], ob_ps[:], bout_sb[:], op=OP.add)
                nc.sync.dma_start(p_out[b], ob[:])
    nc.compile()
    return nc


def prep_inputs(condition, Wih_enc, Whh_enc, bih_enc, bhh_enc,
                Wih_cell, Whh_cell, bih_cell, bhh_cell, W_out, b_out,
                n_steps=N_STEPS):
    def tile64(Wt):  # [512, 2048] -> [128, 64*128], col block m*4+k
        return np.ascontiguousarray(
            Wt.reshape(KH, 128, M16, 128).transpose(1, 2, 0, 3).reshape(128, 64 * 128)).astype(BF)

    whhT = tile64(Whh_enc.T.astype(np.float32))
    wcellT = tile64(Wih_cell.T.astype(np.float32))
    wih_a = np.zeros((C + 1, G), np.float32)
    wih_a[0:C] = Wih_enc.T
    wih_a[C] = bih_enc + bhh_enc
    bias_c = (bih_cell + bhh_cell).astype(np.float32)
    bcell_bc = np.repeat(bias_c.reshape(M16, 128).T[:, :, None], B, axis=2).reshape(128, 256)
    woutT = np.ascontiguousarray(
        W_out.T.reshape(8, 128, C).transpose(1, 0, 2).reshape(128, 8 * C)).astype(BF)
    bout_bc = np.tile(b_out[None, :].astype(np.float32), (T, 1))
    ident = np.eye(128, dtype=np.float32).astype(BF)

    shared = {
        "wihT_a": wih_a.astype(BF), "whhT": whhT, "wcellT": wcellT,
        "bcell_bc": bcell_bc.astype(np.float32), "woutT": woutT,
        "bout_bc": bout_bc, "ident": ident,
    }
    maps = []
    for core in range(NCORES):
        cb = condition[core * B:(core + 1) * B, :n_steps, :]  # [16, n, 32]
        ca = np.ones((C + 1, n_steps * B), np.float32)
        ca[0:C] = cb.transpose(2, 1, 0).reshape(C, n_steps * B)  # col = n*16+b
        m = dict(shared)
        m["cond_aT"] = ca.astype(BF)
        maps.append(m)
    return maps


_NC_CACHE = {}
LAST_RESULT = None


def _ensure_ntff_hook():
    """The agent image's antenv lacks axon_hooks; provide it and register the
    ctypes NTFF profiling hook so trace=True works under axon."""
    import types
    if "antenv.axon_hooks" in sys.modules:
        return
    mod = types.ModuleType("antenv.axon_hooks")
    _h = [None]
    mod.set_axon_ntff_profile_hook = lambda h: _h.__setitem__(0, h)
    mod.get_axon_ntff_profile_hook = lambda: _h[0]
    sys.modules["antenv.axon_hooks"] = mod
    if "/root/.axon_site" not in sys.path:
        sys.path.insert(0, "/root/.axon_site")
    from trn_agent_boot.trn_boot import _ntff_profile_via_ctypes
    mod.set_axon_ntff_profile_hook(_ntff_profile_via_ctypes("/opt/axon/libaxon_pjrt.so"))


def kernel(_trace=False, **inputs):
    global LAST_RESULT
    if _trace:
        try:
            _ensure_ntff_hook()
        except Exception as e:
            print("ntff hook setup failed:", e)
    inputs = {k: np.asarray(v) for k, v in inputs.items()}
    n_steps = N_STEPS
    if n_steps not in _NC_CACHE:
        _NC_CACHE[n_steps] = build_program(n_steps)
    nc = _NC_CACHE[n_steps]
    maps = prep_inputs(**inputs, n_steps=n_steps)
    res = run_bass_kernel_spmd(nc, maps, list(range(NCORES)), trace=_trace)
    LAST_RESULT = res
    out = np.concatenate([np.asarray(res.results[i]["out"], dtype=np.float32)
                          for i in range(NCORES)], axis=0)
    return out



# revision 11
# speedup vs baseline: 1.5556x; 1.5556x over previous
"""Trainium2 Bass kernel for nn_CCGGenerator (LSTM encoder + attention decoder).

Sharding: data-parallel, batch 128 -> 16 per core across 8 cores.
All weights replicated. Self-contained; everything hardcoded.

v2 design (per core, B=16):
- Encoder gates computed transposed: gates.T [2048, 16] as 16 PSUM cols-of-16
  split into TWO per-group PSUM tiles [128, 128] (group g = hidden half).
  Gate m-chunk order is re-packed in prep to
    [G0: i0 i1 f0 f1 o0 o1 g0 g1 | G1: i2 i3 f2 f3 o2 o3 g2 g3]
  so each group's gates are contiguous 128 PSUM cols: [i(32) f(32) o(32) g(32)].
- g-gate rows are pre-scaled x2 so tanh(x) = 2*sigmoid(2x)-1 turns the whole
  gate nonlinearity into ONE sigmoid over 128 cols + a tiny affine on 32 cols.
- Tail per group (8 wide ops instead of ~23 narrow ones):
    DVE  add   ga = gps_g + xg_g            [128,128]
    ACT  sigm  ga = sigmoid(ga)             [128,128] (g cols pre-scaled)
    DVE  affn  gt = 2*sigm-1                [128,32]
    DVE  ig    = i * gt                     [128,32]
    GPS  fc    = f * c_prev                 [128,32]   (parallel w/ DVE)
    DVE  c     = ig + fc                    [128,32]
    ACT  th    = tanh(c)                    [128,32]
    DVE  h     = o * th -> cl_sb (bf16)     [128,32]
- Software pipeline: PE emits S1=G0 MMs, S2=G1 MMs; tail(G0) overlaps S2 and
  next step's S1 (which only needs h chunks k=0,1 produced by tail(G0)).
- xg = x @ Wih.T + bias precomputed per 8-step chunk, stored s-major
  [128, s*256 + m*16 + b] so the tail add is a flat 2-level AP.
- Decoder: same structure, c = i*g (h0=c0=0 per reference), bias instead of xg.
- Attention per b unchanged from v1 (scores via strided CL reads, PE
  transposes for CL_b, ctx matmul, fused leaky-relu output GEMM).
"""
import sys
sys.path.insert(0, "/opt/trn_rl_repo")

import numpy as np
import ml_dtypes
from contextlib import ExitStack

import concourse.bass as bass
import concourse.tile as tile
from concourse import bacc, mybir
from concourse.bass_utils import run_bass_kernel_spmd

F32 = mybir.dt.float32
BF16 = mybir.dt.bfloat16
AF = mybir.ActivationFunctionType
OP = mybir.AluOpType
BF = ml_dtypes.bfloat16

NCORES = 8
B = 16          # batch per core
N_STEPS = 1024  # encoder sequence length
SCH = 8         # steps per xg chunk
C = 32
H = 512
G = 2048        # 4H
T = 24
KH = 4          # hidden 128-chunks
M16 = 16        # gate-dim 128-chunks


def build_program(n_steps=N_STEPS):
    nch = n_steps // SCH
    nac = n_steps // 128  # attention n-chunks
    nc = bacc.Bacc("TRN2", target_bir_lowering=False, debug=False,
                   num_devices=NCORES)

    p_cond = nc.declare_dram_parameter("cond_aT", [C + 1, n_steps * B], BF16, isOutput=False)
    p_wih = nc.declare_dram_parameter("wihT_a", [C + 1, G], BF16, isOutput=False)
    p_whh = nc.declare_dram_parameter("whhT", [128, 64 * 128], BF16, isOutput=False)
    p_wcell = nc.declare_dram_parameter("wcellT", [128, 64 * 128], BF16, isOutput=False)
    p_bcell = nc.declare_dram_parameter("bcell_bc", [128, 256], F32, isOutput=False)
    p_wout = nc.declare_dram_parameter("woutT", [128, 8 * C], BF16, isOutput=False)
    p_bout = nc.declare_dram_parameter("bout_bc", [T, C], F32, isOutput=False)
    p_id = nc.declare_dram_parameter("ident", [128, 128], BF16, isOutput=False)
    p_out = nc.declare_dram_parameter("out", [B, T, C], F32, isOutput=True)

    with tile.TileContext(nc) as tc, ExitStack() as ctx:
        const = ctx.enter_context(tc.tile_pool(name="const", bufs=1))

        cl_sb = const.tile([128, n_steps * KH * B], BF16, tag="cl_sb")  # 16 MB
        hd_sb = const.tile([128, T * KH * B], BF16, tag="hd_sb")       # t-major
        wout_sb = const.tile([128, 8 * C], BF16, tag="wout_sb")
        nc.sync.dma_start(wout_sb[:], p_wout[:])
        bout_sb = const.tile([T, C], F32, tag="bout_sb")
        nc.sync.dma_start(bout_sb[:], p_bout[:])
        id_sb = const.tile([128, 128], BF16, tag="id_sb")
        nc.sync.dma_start(id_sb[:], p_id[:])
        bcell_sb = const.tile([128, 256], F32, tag="bcell_sb")
        nc.sync.dma_start(bcell_sb[:], p_bcell[:])

        cl3 = cl_sb[:].rearrange("p (n k b) -> p n k b", k=KH, b=B)
        hd_v = hd_sb[:].rearrange("p (t k b) -> p t k b", k=KH, b=B)

        def lstm_tail(g, gps_g, xg_g, c_f, h_out, st_pool, ga_pool, fc_pool):
            """Per-group tail. gps_g [128,128] PSUM (cols i f o gt, 32 each).
            xg_g: [128,128] AP to add (xg slice or bias slice).
            c_f: persistent cell state [128, 64] or None (decoder: c = i*g).
            h_out: [128,32] AP (bf16 dest)."""
            ga = ga_pool.tile([128, 128], F32, tag=f"ga{g}")
            nc.vector.tensor_tensor(ga[:], gps_g[:, 0:128], xg_g, op=OP.add)
            nc.vector.tensor_tensor(ga[:], ga[:], gps_g[:, 128:256], op=OP.add)
            nc.scalar.activation(ga[:, 0:96], ga[:, 0:96], AF.Sigmoid)
            nc.scalar.activation(ga[:, 96:128], ga[:, 96:128], AF.Tanh)
            ig = st_pool.tile([128, 32], F32, tag=f"ig{g}")
            nc.vector.tensor_tensor(ig[:], ga[:, 0:32], ga[:, 96:128], op=OP.mult)
            if c_f is not None:
                fc = fc_pool.tile([128, 32], F32, tag=f"fc{g}")
                nc.gpsimd.tensor_tensor(fc[:], ga[:, 32:64],
                                        c_f[:, g * 32:(g + 1) * 32], op=OP.mult)
                nc.vector.tensor_tensor(c_f[:, g * 32:(g + 1) * 32], ig[:], fc[:],
                                        op=OP.add)
                csrc = c_f[:, g * 32:(g + 1) * 32]
            else:
                csrc = ig[:]
            th = st_pool.tile([128, 32], F32, tag=f"th{g}")
            nc.scalar.activation(th[:], csrc, AF.Tanh)
            nc.vector.tensor_tensor(h_out, ga[:, 64:96], th[:], op=OP.mult)

        # ---------------- encoder + decoder (shared gate/state pools) ----------------
        rec_pools = ExitStack()
        gps_pool = rec_pools.enter_context(tc.tile_pool(name="gps", bufs=2, space="PSUM"))
        ga_pool = rec_pools.enter_context(tc.tile_pool(name="gtmp", bufs=2))
        st_pool = rec_pools.enter_context(tc.tile_pool(name="st", bufs=2))
        fc_pool = rec_pools.enter_context(tc.tile_pool(name="fcp", bufs=2))

        def step_mms(gps2, whh_like, hin):
            """Emit the 64 recurrence MMs for one step, grouped G0 then G1.
            Each gate m-chunk accumulates as TWO complete 2-MM psum groups
            (k{0,1} partial in cols 0:128, k{2,3} partial in cols 128:256) so
            no psum bank ever holds two open accumulation groups, while the
            k{0,1} block of step t+1 still only depends on h chunks 0,1."""
            for g in (0, 1):
                for half in (0, 1):
                    for mj in range(8):
                        m = g * 8 + mj
                        for k in (2 * half, 2 * half + 1):
                            nc.tensor.matmul(
                                gps2[g][:, half * 128 + mj * B:half * 128 + (mj + 1) * B],
                                whh_like[:, bass.ts(m * KH + k, 128)],
                                hin(k), start=(k % 2 == 0), stop=(k % 2 == 1))

        with tc.tile_pool(name="enc", bufs=1) as enc_pool, \
             tc.tile_pool(name="xg", bufs=2) as xg_pool, \
             tc.tile_pool(name="xg_ps", bufs=2, space="PSUM") as xg_ps_pool:
            whh_sb = enc_pool.tile([128, 64 * 128], BF16, tag="whh_sb")
            nc.sync.dma_start(whh_sb[:], p_whh[:])
            wih_sb = enc_pool.tile([C + 1, G], BF16, tag="wih_sb")
            nc.sync.dma_start(wih_sb[:], p_wih[:])
            c_f = enc_pool.tile([128, KH * B], F32, tag="c_f")
            h0 = enc_pool.tile([128, KH * B], BF16, tag="h0")
            nc.any.memset(c_f[:], 0.0)
            nc.any.memset(h0[:], 0.0)

            def fetch_cond(j):
                t = xg_pool.tile([C + 1, SCH * B], BF16, tag="cond_ch")
                nc.sync.dma_start(t[:], p_cond[:, bass.ts(j, SCH * B)])
                return t

            def xg_alloc():
                t = xg_pool.tile([128, SCH * M16 * B], F32, tag="xg_sb")
                return t, t[:].rearrange("p (s m b) -> p s m b", s=SCH, m=M16)

            def xg_compute(xg_v, cond_t, mpair):
                """Two gate m-chunks (2*mpair, 2*mpair+1) of xg for a whole
                8-step chunk: two MMs into one PSUM tile, one DVE copy out
                (GPSIMD cannot read PSUM)."""
                xps = xg_ps_pool.tile([128, 2 * SCH * B], F32, tag="xps")
                for i in (0, 1):
                    nc.tensor.matmul(xps[:, bass.ts(i, SCH * B)],
                                     wih_sb[:, bass.ts(2 * mpair + i, 128)],
                                     cond_t[:], start=True, stop=True)
                nc.vector.tensor_copy(
                    xg_v[:, :, 2 * mpair:2 * mpair + 2, :],
                    xps[:].rearrange("p (m s b) -> p s m b", m=2, s=SCH))

            # prologue: cond 0,1 in flight; xg[0] computed in a burst
            cond_cur = fetch_cond(0)
            cond_nxt = fetch_cond(1) if nch > 1 else None
            xg_cur, xgv_cur = xg_alloc()
            for mp in range(M16 // 2):
                xg_compute(xgv_cur, cond_cur, mp)

            for chv in range(nch):
                if chv + 2 < nch:
                    cond_fut = fetch_cond(chv + 2)
                else:
                    cond_fut = None
                if chv + 1 < nch:
                    xg_nxt, xgv_nxt = xg_alloc()
                for s in range(SCH):
                    n = chv * SCH + s
                    # interleave next chunk's xg precompute: 2 m-chunks/step
                    if chv + 1 < nch:
                        xg_compute(xgv_nxt, cond_nxt, s)
                    hin = (lambda k: h0[:, bass.ts(k, B)]) if n == 0 else \
                        (lambda k, _n=n: cl_sb[:, (_n - 1) * 64 + k * B:(_n - 1) * 64 + (k + 1) * B])
                    gps2 = [gps_pool.tile([128, 256], F32, tag=f"gps{g}",
                                           name=f"gps{g}") for g in (0, 1)]
                    step_mms(gps2, whh_sb, hin)
                    for g in (0, 1):
                        lstm_tail(g, gps2[g][:], xg_cur[:, s * 256 + g * 128:s * 256 + (g + 1) * 128],
                                  c_f,
                                  cl_sb[:, n * 64 + g * 32:n * 64 + (g + 1) * 32],
                                  st_pool, ga_pool, fc_pool)
                if chv + 1 < nch:
                    xg_cur, xgv_cur = xg_nxt, xgv_nxt
                    cond_cur, cond_nxt = cond_nxt, cond_fut

        # ---------------- decoder ----------------
        with tc.tile_pool(name="dec", bufs=1) as dec_pool:
            wcell_sb = dec_pool.tile([128, 64 * 128], BF16, tag="wcell_sb")
            nc.sync.dma_start(wcell_sb[:], p_wcell[:])
            for t in range(T):
                if t == 0:
                    hin = lambda k: cl_sb[:, (n_steps - 1) * 64 + k * B:(n_steps - 1) * 64 + (k + 1) * B]
                else:
                    hin = lambda k, _t=t: hd_sb[:, (_t - 1) * 64 + k * B:(_t - 1) * 64 + (k + 1) * B]
                gps2 = [gps_pool.tile([128, 256], F32, tag=f"gps{g}",
                                       name=f"gps{g}") for g in (0, 1)]
                step_mms(gps2, wcell_sb, hin)
                for g in (0, 1):
                    lstm_tail(g, gps2[g][:], bcell_sb[:, g * 128:(g + 1) * 128],
                              None,
                              hd_sb[:, t * 64 + g * 32:t * 64 + (g + 1) * 32],
                              st_pool, ga_pool, fc_pool)
        rec_pools.close()

        # ---------------- attention + output, per batch ----------------
        with tc.tile_pool(name="att_fix", bufs=2) as att_fix, \
             tc.tile_pool(name="scr_ps", bufs=1, space="PSUM") as scr_ps_pool, \
             tc.tile_pool(name="tp_ps", bufs=2, space="PSUM") as tp_ps_pool, \
             tc.tile_pool(name="ctx_ps", bufs=2, space="PSUM") as ctx_ps_pool:
            for b in range(B):
                # scores [24, n]: lhsT = hd strided, rhs = cl_sb strided (CL.T native)
                scr = scr_ps_pool.tile([T, n_steps], F32, tag="scr")
                scn = min(512, n_steps)
                for k in range(KH):
                    for j in range(n_steps // scn):
                        rhs = cl3[:, j * scn:(j + 1) * scn, k, b]
                        nc.tensor.matmul(scr[:, bass.ts(j, scn)], hd_v[:, :, k, b],
                                         rhs, start=(k == 0), stop=(k == KH - 1))
                nmx = att_fix.tile([T, 1], F32, tag="nmx")
                nc.vector.reduce_max(nmx[:], scr[:], axis=mybir.AxisListType.X, negate=True)
                ex = att_fix.tile([T, n_steps], F32, tag="ex")
                sm = att_fix.tile([T, 1], F32, tag="sm")
                nc.scalar.activation(ex[:], scr[:], AF.Exp, bias=nmx[:], accum_out=sm[:])
                rc = att_fix.tile([T, 1], F32, tag="rc")
                nc.vector.reciprocal(rc[:], sm[:])
                cof = att_fix.tile([T, n_steps], BF16, tag="cof")
                nc.vector.tensor_scalar(cof[:], ex[:], rc[:], None, op0=OP.mult)
                # coeff.T [n, 24] via PE transposes
                cT = att_fix.tile([128, nac * T], BF16, tag="cT")
                for j in range(nac):
                    tp = tp_ps_pool.tile([128, 128], BF16, tag="tp")
                    nc.tensor.transpose(tp[:, 0:T], cof[:, bass.ts(j, 128)], id_sb[0:T, 0:T])
                    nc.vector.tensor_copy(cT[:, bass.ts(j, T)], tp[:, 0:T])
                # CL_b n-partitioned tiles via PE transposes
                clb = att_fix.tile([128, nac * KH * 128], BF16, tag="clb")
                for j in range(nac):
                    for k in range(KH):
                        tpc = tp_ps_pool.tile([128, 128], BF16, tag="tp")
                        nc.tensor.transpose(tpc[:], cl3[:, j * 128:(j + 1) * 128, k, b],
                                            id_sb[:, :])
                        nc.vector.tensor_copy(clb[:, bass.ts(j * KH + k, 128)], tpc[:])
                # ctx.T [512, 24]
                ctxp = ctx_ps_pool.tile([128, KH * T], F32, tag="ctxp")
                for k in range(KH):
                    for j in range(nac):
                        nc.tensor.matmul(ctxp[:, bass.ts(k, T)],
                                         clb[:, bass.ts(j * KH + k, 128)],
                                         cT[:, bass.ts(j, T)],
                                         start=(j == 0), stop=(j == nac - 1))
                # out [24, 32]
                ob_ps = scr_ps_pool.tile([T, C], F32, tag="ob_ps")
                for jj in range(8):
                    lr = att_fix.tile([128, T], BF16, tag="lr")
                    src = hd_v[:, :, jj, b] if jj < KH else ctxp[:, bass.ts(jj - KH, T)]
                    nc.scalar.activation(lr[:], src, AF.Lrelu, alpha=0.01)
                    nc.tensor.matmul(ob_ps[:], lr[:], wout_sb[:, bass.ts(jj, C)],
                                     start=(jj == 0), stop=(jj == 7))
                ob = att_fix.tile([T, C], F32, tag="ob")
                nc.vector.tensor_tensor(ob[:], ob_ps[:], bout_sb[:], op=OP.add)
                nc.sync.dma_start(p_out[b], ob[:])
    nc.compile()
    return nc


# m-chunk permutation: new m-position -> original gate chunk index (of 16).
# Original chunks: i=0..3, f=4..7, g=8..11, o=12..15 (PyTorch i,f,g,o order).
# New order: [G0: i0 i1 f0 f1 o0 o1 g0 g1 | G1: i2 i3 f2 f3 o2 o3 g2 g3]
M_PERM = [0, 1, 4, 5, 12, 13, 8, 9,
          2, 3, 6, 7, 14, 15, 10, 11]


def _reorder_cols(Wt):
    """Wt [*, 2048]: permute gate columns into the new m-chunk order."""
    Wn = Wt.reshape(Wt.shape[0], 16, 128)[:, M_PERM, :]
    return np.ascontiguousarray(Wn).reshape(Wt.shape[0], 2048)


def prep_inputs(condition, Wih_enc, Whh_enc, bih_enc, bhh_enc,
                Wih_cell, Whh_cell, bih_cell, bhh_cell, W_out, b_out,
                n_steps=N_STEPS):
    def tile64(Wt):  # [512, 2048] -> [128, 64*128], col block m*4+k
        return np.ascontiguousarray(
            Wt.reshape(KH, 128, M16, 128).transpose(1, 2, 0, 3).reshape(128, 64 * 128)).astype(BF)

    whhT = tile64(_reorder_cols(Whh_enc.T.astype(np.float32)))
    wcellT = tile64(_reorder_cols(Wih_cell.T.astype(np.float32)))
    wih_a = np.zeros((C + 1, G), np.float32)
    wih_a[0:C] = Wih_enc.T
    wih_a[C] = bih_enc + bhh_enc
    wih_a = _reorder_cols(wih_a)
    bias_c = _reorder_cols((bih_cell + bhh_cell).astype(np.float32)[None, :])[0]
    bcell_bc = np.repeat(bias_c.reshape(M16, 128).T[:, :, None], B, axis=2).reshape(128, 256)
    woutT = np.ascontiguousarray(
        W_out.T.reshape(8, 128, C).transpose(1, 0, 2).reshape(128, 8 * C)).astype(BF)
    bout_bc = np.tile(b_out[None, :].astype(np.float32), (T, 1))
    ident = np.eye(128, dtype=np.float32).astype(BF)

    shared = {
        "wihT_a": wih_a.astype(BF), "whhT": whhT, "wcellT": wcellT,
        "bcell_bc": bcell_bc.astype(np.float32), "woutT": woutT,
        "bout_bc": bout_bc, "ident": ident,
    }
    maps = []
    for core in range(NCORES):
        cb = condition[core * B:(core + 1) * B, :n_steps, :]  # [16, n, 32]
        ca = np.ones((C + 1, n_steps * B), np.float32)
        ca[0:C] = cb.transpose(2, 1, 0).reshape(C, n_steps * B)  # col = n*16+b
        m = dict(shared)
        m["cond_aT"] = ca.astype(BF)
        maps.append(m)
    return maps


_NC_CACHE = {}
LAST_RESULT = None


def _ensure_ntff_hook():
    """The agent image's antenv lacks axon_hooks; provide it and register the
    ctypes NTFF profiling hook so trace=True works under axon."""
    import types
    if "antenv.axon_hooks" in sys.modules:
        return
    mod = types.ModuleType("antenv.axon_hooks")
    _h = [None]
    mod.set_axon_ntff_profile_hook = lambda h: _h.__setitem__(0, h)
    mod.get_axon_ntff_profile_hook = lambda: _h[0]
    sys.modules["antenv.axon_hooks"] = mod
    if "/root/.axon_site" not in sys.path:
        sys.path.insert(0, "/root/.axon_site")
    from trn_agent_boot.trn_boot import _ntff_profile_via_ctypes
    mod.set_axon_ntff_profile_hook(_ntff_profile_via_ctypes("/opt/axon/libaxon_pjrt.so"))


def kernel(_trace=False, **inputs):
    global LAST_RESULT
    if _trace:
        try:
            _ensure_ntff_hook()
        except Exception as e:
            print("ntff hook setup failed:", e)
    inputs = {k: np.asarray(v) for k, v in inputs.items()}
    n_steps = N_STEPS
    if n_steps not in _NC_CACHE:
        _NC_CACHE[n_steps] = build_program(n_steps)
    nc = _NC_CACHE[n_steps]
    maps = prep_inputs(**inputs, n_steps=n_steps)
    res = run_bass_kernel_spmd(nc, maps, list(range(NCORES)), trace=_trace)
    LAST_RESULT = res
    out = np.concatenate([np.asarray(res.results[i]["out"], dtype=np.float32)
                          for i in range(NCORES)], axis=0)
    return out


# revision 12
# speedup vs baseline: 1.7368x; 1.1165x over previous
"""Trainium2 Bass kernel for nn_CCGGenerator (LSTM encoder + attention decoder).

Sharding: data-parallel, batch 128 -> 16 per core across 8 cores.
All weights replicated. Self-contained; everything hardcoded.

v2 design (per core, B=16):
- Encoder gates computed transposed: gates.T [2048, 16] as 16 PSUM cols-of-16
  split into TWO per-group PSUM tiles [128, 128] (group g = hidden half).
  Gate m-chunk order is re-packed in prep to
    [G0: i0 i1 f0 f1 o0 o1 g0 g1 | G1: i2 i3 f2 f3 o2 o3 g2 g3]
  so each group's gates are contiguous 128 PSUM cols: [i(32) f(32) o(32) g(32)].
- g-gate rows are pre-scaled x2 so tanh(x) = 2*sigmoid(2x)-1 turns the whole
  gate nonlinearity into ONE sigmoid over 128 cols + a tiny affine on 32 cols.
- Tail per group (8 wide ops instead of ~23 narrow ones):
    DVE  add   ga = gps_g + xg_g            [128,128]
    ACT  sigm  ga = sigmoid(ga)             [128,128] (g cols pre-scaled)
    DVE  affn  gt = 2*sigm-1                [128,32]
    DVE  ig    = i * gt                     [128,32]
    GPS  fc    = f * c_prev                 [128,32]   (parallel w/ DVE)
    DVE  c     = ig + fc                    [128,32]
    ACT  th    = tanh(c)                    [128,32]
    DVE  h     = o * th -> cl_sb (bf16)     [128,32]
- Software pipeline: PE emits S1=G0 MMs, S2=G1 MMs; tail(G0) overlaps S2 and
  next step's S1 (which only needs h chunks k=0,1 produced by tail(G0)).
- xg = x @ Wih.T + bias precomputed per 8-step chunk, stored s-major
  [128, s*256 + m*16 + b] so the tail add is a flat 2-level AP.
- Decoder: same structure, c = i*g (h0=c0=0 per reference), bias instead of xg.
- Attention per b unchanged from v1 (scores via strided CL reads, PE
  transposes for CL_b, ctx matmul, fused leaky-relu output GEMM).
"""
import sys
sys.path.insert(0, "/opt/trn_rl_repo")

import numpy as np
import ml_dtypes
from contextlib import ExitStack

import concourse.bass as bass
import concourse.tile as tile
from concourse import bacc, mybir
from concourse.bass_utils import run_bass_kernel_spmd

F32 = mybir.dt.float32
BF16 = mybir.dt.bfloat16
AF = mybir.ActivationFunctionType
OP = mybir.AluOpType
BF = ml_dtypes.bfloat16

NCORES = 8
B = 16          # batch per core
N_STEPS = 1024  # encoder sequence length
SCH = 8         # steps per xg chunk
C = 32
H = 512
G = 2048        # 4H
T = 24
KH = 4          # hidden 128-chunks
M16 = 16        # gate-dim 128-chunks


def build_program(n_steps=N_STEPS):
    nch = n_steps // SCH
    nac = n_steps // 128  # attention n-chunks
    nc = bacc.Bacc("TRN2", target_bir_lowering=False, debug=False,
                   num_devices=NCORES)

    p_cond = nc.declare_dram_parameter("cond_aT", [C + 1, n_steps * B], BF16, isOutput=False)
    p_wih = nc.declare_dram_parameter("wihT_a", [C + 1, G], BF16, isOutput=False)
    p_whh = nc.declare_dram_parameter("whhT", [128, 64 * 128], BF16, isOutput=False)
    p_wcell = nc.declare_dram_parameter("wcellT", [128, 64 * 128], BF16, isOutput=False)
    p_bcell = nc.declare_dram_parameter("bcell_bc", [128, 256], F32, isOutput=False)
    p_wout = nc.declare_dram_parameter("woutT", [128, 8 * C], BF16, isOutput=False)
    p_bout = nc.declare_dram_parameter("bout_bc", [T, C], F32, isOutput=False)
    p_id = nc.declare_dram_parameter("ident", [128, 128], BF16, isOutput=False)
    p_out = nc.declare_dram_parameter("out", [B, T, C], F32, isOutput=True)

    with tile.TileContext(nc) as tc, ExitStack() as ctx:
        const = ctx.enter_context(tc.tile_pool(name="const", bufs=1))

        cl_sb = const.tile([128, n_steps * KH * B], BF16, tag="cl_sb")  # 16 MB
        hd_sb = const.tile([128, T * KH * B], BF16, tag="hd_sb")       # t-major
        wout_sb = const.tile([128, 8 * C], BF16, tag="wout_sb")
        nc.sync.dma_start(wout_sb[:], p_wout[:])
        bout_sb = const.tile([T, C], F32, tag="bout_sb")
        nc.sync.dma_start(bout_sb[:], p_bout[:])
        id_sb = const.tile([128, 128], BF16, tag="id_sb")
        nc.sync.dma_start(id_sb[:], p_id[:])
        bcell_sb = const.tile([128, 256], F32, tag="bcell_sb")
        nc.sync.dma_start(bcell_sb[:], p_bcell[:])

        cl3 = cl_sb[:].rearrange("p (n k b) -> p n k b", k=KH, b=B)
        hd_v = hd_sb[:].rearrange("p (t k b) -> p t k b", k=KH, b=B)

        def lstm_tail(g, gps_g, xg_g, c_f, h_out, st_pool, ga_pool, fc_pool):
            """Per-group tail. gps_g [128,128] PSUM (cols i f o gt, 32 each).
            xg_g: [128,128] AP to add (xg slice or bias slice).
            c_f: persistent cell state [128, 64] or None (decoder: c = i*g).
            h_out: [128,32] AP (bf16 dest)."""
            ga = ga_pool.tile([128, 128], F32, tag=f"ga{g}")
            nc.vector.tensor_tensor(ga[:], gps_g[0][:], xg_g, op=OP.add)
            nc.vector.tensor_tensor(ga[:], ga[:], gps_g[1][:], op=OP.add)
            nc.scalar.activation(ga[:], ga[:], AF.Sigmoid)
            nc.vector.tensor_scalar(ga[:, 96:128], ga[:, 96:128], 2.0, -1.0,
                                    op0=OP.mult, op1=OP.add)
            ig = st_pool.tile([128, 32], F32, tag=f"ig{g}")
            nc.vector.tensor_tensor(ig[:], ga[:, 0:32], ga[:, 96:128], op=OP.mult)
            if c_f is not None:
                fc = fc_pool.tile([128, 32], F32, tag=f"fc{g}")
                nc.gpsimd.tensor_tensor(fc[:], ga[:, 32:64],
                                        c_f[:, g * 32:(g + 1) * 32], op=OP.mult)
                nc.vector.tensor_tensor(c_f[:, g * 32:(g + 1) * 32], ig[:], fc[:],
                                        op=OP.add)
                csrc = c_f[:, g * 32:(g + 1) * 32]
            else:
                csrc = ig[:]
            th = st_pool.tile([128, 32], F32, tag=f"th{g}")
            nc.scalar.activation(th[:], csrc, AF.Tanh)
            nc.vector.tensor_tensor(h_out, ga[:, 64:96], th[:], op=OP.mult)

        # ---------------- encoder + decoder (shared gate/state pools) ----------------
        rec_pools = ExitStack()
        gps_pool = rec_pools.enter_context(tc.tile_pool(name="gps", bufs=1, space="PSUM"))
        ga_pool = rec_pools.enter_context(tc.tile_pool(name="gtmp", bufs=2))
        st_pool = rec_pools.enter_context(tc.tile_pool(name="st", bufs=2))
        fc_pool = rec_pools.enter_context(tc.tile_pool(name="fcp", bufs=2))

        def step_mms(gps2, whh_like, hin):
            """Emit the 64 recurrence MMs for one step, grouped G0 then G1.
            Each gate m-chunk accumulates as TWO complete 2-MM psum groups:
            k{0,1} partial in pa, k{2,3} partial in pb (separate banks, so the
            tail's pa read never bank-conflicts with pb writes and no bank
            holds two open accumulation groups). The k{0,1} block of step t+1
            only depends on h chunks 0,1 (tail g0 of step t)."""
            for g in (0, 1):
                for half in (0, 1):
                    for mj in range(8):
                        m = g * 8 + mj
                        for k in (2 * half, 2 * half + 1):
                            nc.tensor.matmul(
                                gps2[g][half][:, bass.ts(mj, B)],
                                whh_like[:, bass.ts(m * KH + k, 128)],
                                hin(k), start=(k % 2 == 0), stop=(k % 2 == 1))

        with tc.tile_pool(name="enc", bufs=1) as enc_pool, \
             tc.tile_pool(name="xg", bufs=2) as xg_pool, \
             tc.tile_pool(name="xg_ps", bufs=2, space="PSUM") as xg_ps_pool:
            whh_sb = enc_pool.tile([128, 64 * 128], BF16, tag="whh_sb")
            nc.sync.dma_start(whh_sb[:], p_whh[:])
            wih_sb = enc_pool.tile([C + 1, G], BF16, tag="wih_sb")
            nc.sync.dma_start(wih_sb[:], p_wih[:])
            c_f = enc_pool.tile([128, KH * B], F32, tag="c_f")
            h0 = enc_pool.tile([128, KH * B], BF16, tag="h0")
            nc.any.memset(c_f[:], 0.0)
            nc.any.memset(h0[:], 0.0)

            def fetch_cond(j):
                t = xg_pool.tile([C + 1, SCH * B], BF16, tag="cond_ch")
                nc.sync.dma_start(t[:], p_cond[:, bass.ts(j, SCH * B)])
                return t

            def xg_alloc():
                t = xg_pool.tile([128, SCH * M16 * B], F32, tag="xg_sb")
                return t, t[:].rearrange("p (s m b) -> p s m b", s=SCH, m=M16)

            def xg_compute(xg_v, cond_t, mpair):
                """Two gate m-chunks (2*mpair, 2*mpair+1) of xg for a whole
                8-step chunk: two MMs into one PSUM tile, one DVE copy out
                (GPSIMD cannot read PSUM)."""
                xps = xg_ps_pool.tile([128, 2 * SCH * B], F32, tag="xps")
                for i in (0, 1):
                    nc.tensor.matmul(xps[:, bass.ts(i, SCH * B)],
                                     wih_sb[:, bass.ts(2 * mpair + i, 128)],
                                     cond_t[:], start=True, stop=True)
                nc.vector.tensor_copy(
                    xg_v[:, :, 2 * mpair:2 * mpair + 2, :],
                    xps[:].rearrange("p (m s b) -> p s m b", m=2, s=SCH))

            # prologue: cond 0,1 in flight; xg[0] computed in a burst
            cond_cur = fetch_cond(0)
            cond_nxt = fetch_cond(1) if nch > 1 else None
            xg_cur, xgv_cur = xg_alloc()
            for mp in range(M16 // 2):
                xg_compute(xgv_cur, cond_cur, mp)

            for chv in range(nch):
                if chv + 2 < nch:
                    cond_fut = fetch_cond(chv + 2)
                else:
                    cond_fut = None
                if chv + 1 < nch:
                    xg_nxt, xgv_nxt = xg_alloc()
                for s in range(SCH):
                    n = chv * SCH + s
                    hin = (lambda k: h0[:, bass.ts(k, B)]) if n == 0 else \
                        (lambda k, _n=n: cl_sb[:, (_n - 1) * 64 + k * B:(_n - 1) * 64 + (k + 1) * B])
                    gps2 = [[gps_pool.tile([128, 128], F32, tag=f"gp{g}{h_}",
                                            name=f"gp{g}{h_}") for h_ in (0, 1)]
                            for g in (0, 1)]
                    step_mms(gps2, whh_sb, hin)
                    # next chunk's xg precompute: fills the PE's wait-for-h gap
                    if chv + 1 < nch:
                        xg_compute(xgv_nxt, cond_nxt, s)
                    for g in (0, 1):
                        lstm_tail(g, gps2[g], xg_cur[:, s * 256 + g * 128:s * 256 + (g + 1) * 128],
                                  c_f,
                                  cl_sb[:, n * 64 + g * 32:n * 64 + (g + 1) * 32],
                                  st_pool, ga_pool, fc_pool)
                if chv + 1 < nch:
                    xg_cur, xgv_cur = xg_nxt, xgv_nxt
                    cond_cur, cond_nxt = cond_nxt, cond_fut

        # ---------------- decoder ----------------
        with tc.tile_pool(name="dec", bufs=1) as dec_pool:
            wcell_sb = dec_pool.tile([128, 64 * 128], BF16, tag="wcell_sb")
            nc.sync.dma_start(wcell_sb[:], p_wcell[:])
            for t in range(T):
                if t == 0:
                    hin = lambda k: cl_sb[:, (n_steps - 1) * 64 + k * B:(n_steps - 1) * 64 + (k + 1) * B]
                else:
                    hin = lambda k, _t=t: hd_sb[:, (_t - 1) * 64 + k * B:(_t - 1) * 64 + (k + 1) * B]
                gps2 = [[gps_pool.tile([128, 128], F32, tag=f"gp{g}{h_}",
                                        name=f"gp{g}{h_}") for h_ in (0, 1)]
                        for g in (0, 1)]
                step_mms(gps2, wcell_sb, hin)
                for g in (0, 1):
                    lstm_tail(g, gps2[g], bcell_sb[:, g * 128:(g + 1) * 128],
                              None,
                              hd_sb[:, t * 64 + g * 32:t * 64 + (g + 1) * 32],
                              st_pool, ga_pool, fc_pool)
        rec_pools.close()

        # ---------------- attention + output, per batch ----------------
        with tc.tile_pool(name="att_fix", bufs=2) as att_fix, \
             tc.tile_pool(name="scr_ps", bufs=1, space="PSUM") as scr_ps_pool, \
             tc.tile_pool(name="tp_ps", bufs=2, space="PSUM") as tp_ps_pool, \
             tc.tile_pool(name="ctx_ps", bufs=2, space="PSUM") as ctx_ps_pool:
            for b in range(B):
                # scores [24, n]: lhsT = hd strided, rhs = cl_sb strided (CL.T native)
                scr = scr_ps_pool.tile([T, n_steps], F32, tag="scr")
                scn = min(512, n_steps)
                for k in range(KH):
                    for j in range(n_steps // scn):
                        rhs = cl3[:, j * scn:(j + 1) * scn, k, b]
                        nc.tensor.matmul(scr[:, bass.ts(j, scn)], hd_v[:, :, k, b],
                                         rhs, start=(k == 0), stop=(k == KH - 1))
                nmx = att_fix.tile([T, 1], F32, tag="nmx")
                nc.vector.reduce_max(nmx[:], scr[:], axis=mybir.AxisListType.X, negate=True)
                ex = att_fix.tile([T, n_steps], F32, tag="ex")
                sm = att_fix.tile([T, 1], F32, tag="sm")
                nc.scalar.activation(ex[:], scr[:], AF.Exp, bias=nmx[:], accum_out=sm[:])
                rc = att_fix.tile([T, 1], F32, tag="rc")
                nc.vector.reciprocal(rc[:], sm[:])
                cof = att_fix.tile([T, n_steps], BF16, tag="cof")
                nc.vector.tensor_scalar(cof[:], ex[:], rc[:], None, op0=OP.mult)
                # coeff.T [n, 24] via PE transposes
                cT = att_fix.tile([128, nac * T], BF16, tag="cT")
                for j in range(nac):
                    tp = tp_ps_pool.tile([128, 128], BF16, tag="tp")
                    nc.tensor.transpose(tp[:, 0:T], cof[:, bass.ts(j, 128)], id_sb[0:T, 0:T])
                    nc.vector.tensor_copy(cT[:, bass.ts(j, T)], tp[:, 0:T])
                # CL_b n-partitioned tiles via PE transposes
                clb = att_fix.tile([128, nac * KH * 128], BF16, tag="clb")
                for j in range(nac):
                    for k in range(KH):
                        tpc = tp_ps_pool.tile([128, 128], BF16, tag="tp")
                        nc.tensor.transpose(tpc[:], cl3[:, j * 128:(j + 1) * 128, k, b],
                                            id_sb[:, :])
                        nc.vector.tensor_copy(clb[:, bass.ts(j * KH + k, 128)], tpc[:])
                # ctx.T [512, 24]
                ctxp = ctx_ps_pool.tile([128, KH * T], F32, tag="ctxp")
                for k in range(KH):
                    for j in range(nac):
                        nc.tensor.matmul(ctxp[:, bass.ts(k, T)],
                                         clb[:, bass.ts(j * KH + k, 128)],
                                         cT[:, bass.ts(j, T)],
                                         start=(j == 0), stop=(j == nac - 1))
                # out [24, 32]
                ob_ps = scr_ps_pool.tile([T, C], F32, tag="ob_ps")
                for jj in range(8):
                    lr = att_fix.tile([128, T], BF16, tag="lr")
                    src = hd_v[:, :, jj, b] if jj < KH else ctxp[:, bass.ts(jj - KH, T)]
                    nc.scalar.activation(lr[:], src, AF.Lrelu, alpha=0.01)
                    nc.tensor.matmul(ob_ps[:], lr[:], wout_sb[:, bass.ts(jj, C)],
                                     start=(jj == 0), stop=(jj == 7))
                ob = att_fix.tile([T, C], F32, tag="ob")
                nc.vector.tensor_tensor(ob[:], ob_ps[:], bout_sb[:], op=OP.add)
                nc.sync.dma_start(p_out[b], ob[:])
    nc.compile()
    return nc


# m-chunk permutation: new m-position -> original gate chunk index (of 16).
# Original chunks: i=0..3, f=4..7, g=8..11, o=12..15 (PyTorch i,f,g,o order).
# New order: [G0: i0 i1 f0 f1 o0 o1 g0 g1 | G1: i2 i3 f2 f3 o2 o3 g2 g3]
M_PERM = [0, 1, 4, 5, 12, 13, 8, 9,
          2, 3, 6, 7, 14, 15, 10, 11]


def _reorder_cols(Wt):
    """Wt [*, 2048]: permute gate columns into the new m-chunk order and
    pre-scale the g-gate columns by 2 (tanh(x) = 2*sigmoid(2x)-1)."""
    Wn = Wt.reshape(Wt.shape[0], 16, 128)[:, M_PERM, :].copy()
    Wn[:, [6, 7, 14, 15], :] *= 2.0
    return Wn.reshape(Wt.shape[0], 2048)


def prep_inputs(condition, Wih_enc, Whh_enc, bih_enc, bhh_enc,
                Wih_cell, Whh_cell, bih_cell, bhh_cell, W_out, b_out,
                n_steps=N_STEPS):
    def tile64(Wt):  # [512, 2048] -> [128, 64*128], col block m*4+k
        return np.ascontiguousarray(
            Wt.reshape(KH, 128, M16, 128).transpose(1, 2, 0, 3).reshape(128, 64 * 128)).astype(BF)

    whhT = tile64(_reorder_cols(Whh_enc.T.astype(np.float32)))
    wcellT = tile64(_reorder_cols(Wih_cell.T.astype(np.float32)))
    wih_a = np.zeros((C + 1, G), np.float32)
    wih_a[0:C] = Wih_enc.T
    wih_a[C] = bih_enc + bhh_enc
    wih_a = _reorder_cols(wih_a)
    bias_c = _reorder_cols((bih_cell + bhh_cell).astype(np.float32)[None, :])[0]
    bcell_bc = np.repeat(bias_c.reshape(M16, 128).T[:, :, None], B, axis=2).reshape(128, 256)
    woutT = np.ascontiguousarray(
        W_out.T.reshape(8, 128, C).transpose(1, 0, 2).reshape(128, 8 * C)).astype(BF)
    bout_bc = np.tile(b_out[None, :].astype(np.float32), (T, 1))
    ident = np.eye(128, dtype=np.float32).astype(BF)

    shared = {
        "wihT_a": wih_a.astype(BF), "whhT": whhT, "wcellT": wcellT,
        "bcell_bc": bcell_bc.astype(np.float32), "woutT": woutT,
        "bout_bc": bout_bc, "ident": ident,
    }
    maps = []
    for core in range(NCORES):
        cb = condition[core * B:(core + 1) * B, :n_steps, :]  # [16, n, 32]
        ca = np.ones((C + 1, n_steps * B), np.float32)
        ca[0:C] = cb.transpose(2, 1, 0).reshape(C, n_steps * B)  # col = n*16+b
        m = dict(shared)
        m["cond_aT"] = ca.astype(BF)
        maps.append(m)
    return maps


_NC_CACHE = {}
LAST_RESULT = None


def _ensure_ntff_hook():
    """The agent image's antenv lacks axon_hooks; provide it and register the
    ctypes NTFF profiling hook so trace=True works under axon."""
    import types
    if "antenv.axon_hooks" in sys.modules:
        return
    mod = types.ModuleType("antenv.axon_hooks")
    _h = [None]
    mod.set_axon_ntff_profile_hook = lambda h: _h.__setitem__(0, h)
    mod.get_axon_ntff_profile_hook = lambda: _h[0]
    sys.modules["antenv.axon_hooks"] = mod
    if "/root/.axon_site" not in sys.path:
        sys.path.insert(0, "/root/.axon_site")
    from trn_agent_boot.trn_boot import _ntff_profile_via_ctypes
    mod.set_axon_ntff_profile_hook(_ntff_profile_via_ctypes("/opt/axon/libaxon_pjrt.so"))


def kernel(_trace=False, **inputs):
    global LAST_RESULT
    if _trace:
        try:
            _ensure_ntff_hook()
        except Exception as e:
            print("ntff hook setup failed:", e)
    inputs = {k: np.asarray(v) for k, v in inputs.items()}
    n_steps = N_STEPS
    if n_steps not in _NC_CACHE:
        _NC_CACHE[n_steps] = build_program(n_steps)
    nc = _NC_CACHE[n_steps]
    maps = prep_inputs(**inputs, n_steps=n_steps)
    res = run_bass_kernel_spmd(nc, maps, list(range(NCORES)), trace=_trace)
    LAST_RESULT = res
    out = np.concatenate([np.asarray(res.results[i]["out"], dtype=np.float32)
                          for i in range(NCORES)], axis=0)
    return out


# revision 13
# speedup vs baseline: 1.7905x; 1.0309x over previous
"""Trainium2 Bass kernel for nn_CCGGenerator (LSTM encoder + attention decoder).

Sharding: data-parallel, batch 128 -> 16 per core across 8 cores.
All weights replicated. Self-contained; everything hardcoded.

v2 design (per core, B=16):
- Encoder gates computed transposed: gates.T [2048, 16] as 16 PSUM cols-of-16
  split into TWO per-group PSUM tiles [128, 128] (group g = hidden half).
  Gate m-chunk order is re-packed in prep to
    [G0: i0 i1 f0 f1 o0 o1 g0 g1 | G1: i2 i3 f2 f3 o2 o3 g2 g3]
  so each group's gates are contiguous 128 PSUM cols: [i(32) f(32) o(32) g(32)].
- g-gate rows are pre-scaled x2 so tanh(x) = 2*sigmoid(2x)-1 turns the whole
  gate nonlinearity into ONE sigmoid over 128 cols + a tiny affine on 32 cols.
- Tail per group (8 wide ops instead of ~23 narrow ones):
    DVE  add   ga = gps_g + xg_g            [128,128]
    ACT  sigm  ga = sigmoid(ga)             [128,128] (g cols pre-scaled)
    DVE  affn  gt = 2*sigm-1                [128,32]
    DVE  ig    = i * gt                     [128,32]
    GPS  fc    = f * c_prev                 [128,32]   (parallel w/ DVE)
    DVE  c     = ig + fc                    [128,32]
    ACT  th    = tanh(c)                    [128,32]
    DVE  h     = o * th -> cl_sb (bf16)     [128,32]
- Software pipeline: PE emits S1=G0 MMs, S2=G1 MMs; tail(G0) overlaps S2 and
  next step's S1 (which only needs h chunks k=0,1 produced by tail(G0)).
- xg = x @ Wih.T + bias precomputed per 8-step chunk, stored s-major
  [128, s*256 + m*16 + b] so the tail add is a flat 2-level AP.
- Decoder: same structure, c = i*g (h0=c0=0 per reference), bias instead of xg.
- Attention per b unchanged from v1 (scores via strided CL reads, PE
  transposes for CL_b, ctx matmul, fused leaky-relu output GEMM).
"""
import sys
sys.path.insert(0, "/opt/trn_rl_repo")

import numpy as np
import ml_dtypes
from contextlib import ExitStack

import concourse.bass as bass
import concourse.tile as tile
from concourse import bacc, mybir
from concourse.bass_utils import run_bass_kernel_spmd

F32 = mybir.dt.float32
BF16 = mybir.dt.bfloat16
AF = mybir.ActivationFunctionType
OP = mybir.AluOpType
BF = ml_dtypes.bfloat16

NCORES = 8
B = 16          # batch per core
N_STEPS = 1024  # encoder sequence length
SCH = 8         # steps per xg chunk
C = 32
H = 512
G = 2048        # 4H
T = 24
KH = 4          # hidden 128-chunks
M16 = 16        # gate-dim 128-chunks


def build_program(n_steps=N_STEPS):
    nch = n_steps // SCH
    nac = n_steps // 128  # attention n-chunks
    nc = bacc.Bacc("TRN2", target_bir_lowering=False, debug=False,
                   num_devices=NCORES)

    p_cond = nc.declare_dram_parameter("cond_aT", [C + 1, n_steps * B], BF16, isOutput=False)
    p_wih = nc.declare_dram_parameter("wihT_a", [C + 1, G], BF16, isOutput=False)
    p_whh = nc.declare_dram_parameter("whhT", [128, 64 * 128], BF16, isOutput=False)
    p_wcell = nc.declare_dram_parameter("wcellT", [128, 64 * 128], BF16, isOutput=False)
    p_bcell = nc.declare_dram_parameter("bcell_bc", [128, 256], F32, isOutput=False)
    p_wout = nc.declare_dram_parameter("woutT", [128, 8 * C], BF16, isOutput=False)
    p_bout = nc.declare_dram_parameter("bout_bc", [T, C], F32, isOutput=False)
    p_id = nc.declare_dram_parameter("ident", [128, 128], BF16, isOutput=False)
    p_out = nc.declare_dram_parameter("out", [B, T, C], F32, isOutput=True)

    with tile.TileContext(nc) as tc, ExitStack() as ctx:
        const = ctx.enter_context(tc.tile_pool(name="const", bufs=1))

        cl_sb = const.tile([128, n_steps * KH * B], BF16, tag="cl_sb")  # 16 MB
        hd_sb = const.tile([128, T * KH * B], BF16, tag="hd_sb")       # t-major
        wout_sb = const.tile([128, 8 * C], BF16, tag="wout_sb")
        nc.sync.dma_start(wout_sb[:], p_wout[:])
        bout_sb = const.tile([T, C], F32, tag="bout_sb")
        nc.sync.dma_start(bout_sb[:], p_bout[:])
        id_sb = const.tile([128, 128], BF16, tag="id_sb")
        nc.sync.dma_start(id_sb[:], p_id[:])
        bcell_sb = const.tile([128, 256], F32, tag="bcell_sb")
        nc.sync.dma_start(bcell_sb[:], p_bcell[:])

        cl3 = cl_sb[:].rearrange("p (n k b) -> p n k b", k=KH, b=B)
        hd_v = hd_sb[:].rearrange("p (t k b) -> p t k b", k=KH, b=B)

        def lstm_tail(g, gps_g, xg_g, c_f, h_out, st_pool, ga_pool, fc_pool):
            """Per-group tail. gps_g [128,128] PSUM (cols i f o gt, 32 each).
            xg_g: [128,128] AP to add (xg slice or bias slice).
            c_f: persistent cell state [128, 64] or None (decoder: c = i*g).
            h_out: [128,32] AP (bf16 dest)."""
            ga = ga_pool.tile([128, 128], F32, tag=f"ga{g}")
            nc.vector.tensor_tensor(ga[:], gps_g[0][:], xg_g, op=OP.add)
            nc.vector.tensor_tensor(ga[:], ga[:], gps_g[1][:], op=OP.add)
            nc.scalar.activation(ga[:, 0:96], ga[:, 0:96], AF.Sigmoid)
            nc.scalar.activation(ga[:, 96:128], ga[:, 96:128], AF.Tanh)
            # state update on GpSimd: fc, ig, c run back-to-back with no
            # cross-engine hops; DVE only does the psum adds and the h mult.
            ig = st_pool.tile([128, 32], F32, tag=f"ig{g}")
            if c_f is not None:
                fc = fc_pool.tile([128, 32], F32, tag=f"fc{g}")
                nc.gpsimd.tensor_tensor(fc[:], ga[:, 32:64],
                                        c_f[:, g * 32:(g + 1) * 32], op=OP.mult)
                nc.gpsimd.tensor_tensor(ig[:], ga[:, 0:32], ga[:, 96:128], op=OP.mult)
                nc.gpsimd.tensor_tensor(c_f[:, g * 32:(g + 1) * 32], ig[:], fc[:],
                                        op=OP.add)
                csrc = c_f[:, g * 32:(g + 1) * 32]
            else:
                nc.gpsimd.tensor_tensor(ig[:], ga[:, 0:32], ga[:, 96:128], op=OP.mult)
                csrc = ig[:]
            th = st_pool.tile([128, 32], F32, tag=f"th{g}")
            nc.scalar.activation(th[:], csrc, AF.Tanh)
            nc.vector.tensor_tensor(h_out, ga[:, 64:96], th[:], op=OP.mult)

        # ---------------- encoder + decoder (shared gate/state pools) ----------------
        rec_pools = ExitStack()
        gps_pool = rec_pools.enter_context(tc.tile_pool(name="gps", bufs=1, space="PSUM"))
        ga_pool = rec_pools.enter_context(tc.tile_pool(name="gtmp", bufs=2))
        st_pool = rec_pools.enter_context(tc.tile_pool(name="st", bufs=2))
        fc_pool = rec_pools.enter_context(tc.tile_pool(name="fcp", bufs=2))

        def step_mms(gps2, whh_like, hin):
            """Emit the 64 recurrence MMs for one step, grouped G0 then G1.
            Each gate m-chunk accumulates as TWO complete 2-MM psum groups:
            k{0,1} partial in pa, k{2,3} partial in pb (separate banks, so the
            tail's pa read never bank-conflicts with pb writes and no bank
            holds two open accumulation groups). The k{0,1} block of step t+1
            only depends on h chunks 0,1 (tail g0 of step t)."""
            for g in (0, 1):
                for half in (0, 1):
                    for mj in range(8):
                        m = g * 8 + mj
                        for k in (2 * half, 2 * half + 1):
                            nc.tensor.matmul(
                                gps2[g][half][:, bass.ts(mj, B)],
                                whh_like[:, bass.ts(m * KH + k, 128)],
                                hin(k), start=(k % 2 == 0), stop=(k % 2 == 1))

        with tc.tile_pool(name="enc", bufs=1) as enc_pool, \
             tc.tile_pool(name="xg", bufs=2) as xg_pool, \
             tc.tile_pool(name="xg_ps", bufs=2, space="PSUM") as xg_ps_pool:
            whh_sb = enc_pool.tile([128, 64 * 128], BF16, tag="whh_sb")
            nc.sync.dma_start(whh_sb[:], p_whh[:])
            wih_sb = enc_pool.tile([C + 1, G], BF16, tag="wih_sb")
            nc.sync.dma_start(wih_sb[:], p_wih[:])
            c_f = enc_pool.tile([128, KH * B], F32, tag="c_f")
            h0 = enc_pool.tile([128, KH * B], BF16, tag="h0")
            nc.any.memset(c_f[:], 0.0)
            nc.any.memset(h0[:], 0.0)

            def fetch_cond(j):
                t = xg_pool.tile([C + 1, SCH * B], BF16, tag="cond_ch")
                nc.sync.dma_start(t[:], p_cond[:, bass.ts(j, SCH * B)])
                return t

            def xg_alloc():
                t = xg_pool.tile([128, SCH * M16 * B], F32, tag="xg_sb")
                return t, t[:].rearrange("p (s m b) -> p s m b", s=SCH, m=M16)

            def xg_compute(xg_v, cond_t, mpair):
                """Two gate m-chunks (2*mpair, 2*mpair+1) of xg for a whole
                8-step chunk: two MMs into one PSUM tile, one DVE copy out
                (GPSIMD cannot read PSUM)."""
                xps = xg_ps_pool.tile([128, 2 * SCH * B], F32, tag="xps")
                for i in (0, 1):
                    nc.tensor.matmul(xps[:, bass.ts(i, SCH * B)],
                                     wih_sb[:, bass.ts(2 * mpair + i, 128)],
                                     cond_t[:], start=True, stop=True)
                nc.vector.tensor_copy(
                    xg_v[:, :, 2 * mpair:2 * mpair + 2, :],
                    xps[:].rearrange("p (m s b) -> p s m b", m=2, s=SCH))

            # prologue: cond 0,1 in flight; xg[0] computed in a burst
            cond_cur = fetch_cond(0)
            cond_nxt = fetch_cond(1) if nch > 1 else None
            xg_cur, xgv_cur = xg_alloc()
            for mp in range(M16 // 2):
                xg_compute(xgv_cur, cond_cur, mp)

            for chv in range(nch):
                if chv + 2 < nch:
                    cond_fut = fetch_cond(chv + 2)
                else:
                    cond_fut = None
                if chv + 1 < nch:
                    xg_nxt, xgv_nxt = xg_alloc()
                for s in range(SCH):
                    n = chv * SCH + s
                    hin = (lambda k: h0[:, bass.ts(k, B)]) if n == 0 else \
                        (lambda k, _n=n: cl_sb[:, (_n - 1) * 64 + k * B:(_n - 1) * 64 + (k + 1) * B])
                    gps2 = [[gps_pool.tile([128, 128], F32, tag=f"gp{g}{h_}",
                                            name=f"gp{g}{h_}") for h_ in (0, 1)]
                            for g in (0, 1)]
                    step_mms(gps2, whh_sb, hin)
                    # next chunk's xg precompute: fills the PE's wait-for-h gap
                    if chv + 1 < nch:
                        xg_compute(xgv_nxt, cond_nxt, s)
                    for g in (0, 1):
                        lstm_tail(g, gps2[g], xg_cur[:, s * 256 + g * 128:s * 256 + (g + 1) * 128],
                                  c_f,
                                  cl_sb[:, n * 64 + g * 32:n * 64 + (g + 1) * 32],
                                  st_pool, ga_pool, fc_pool)
                if chv + 1 < nch:
                    xg_cur, xgv_cur = xg_nxt, xgv_nxt
                    cond_cur, cond_nxt = cond_nxt, cond_fut

        # ---------------- decoder ----------------
        with tc.tile_pool(name="dec", bufs=1) as dec_pool:
            wcell_sb = dec_pool.tile([128, 64 * 128], BF16, tag="wcell_sb")
            nc.sync.dma_start(wcell_sb[:], p_wcell[:])
            for t in range(T):
                if t == 0:
                    hin = lambda k: cl_sb[:, (n_steps - 1) * 64 + k * B:(n_steps - 1) * 64 + (k + 1) * B]
                else:
                    hin = lambda k, _t=t: hd_sb[:, (_t - 1) * 64 + k * B:(_t - 1) * 64 + (k + 1) * B]
                gps2 = [[gps_pool.tile([128, 128], F32, tag=f"gp{g}{h_}",
                                        name=f"gp{g}{h_}") for h_ in (0, 1)]
                        for g in (0, 1)]
                step_mms(gps2, wcell_sb, hin)
                for g in (0, 1):
                    lstm_tail(g, gps2[g], bcell_sb[:, g * 128:(g + 1) * 128],
                              None,
                              hd_sb[:, t * 64 + g * 32:t * 64 + (g + 1) * 32],
                              st_pool, ga_pool, fc_pool)
        rec_pools.close()

        # ---------------- attention + output, per batch ----------------
        with tc.tile_pool(name="att_fix", bufs=2) as att_fix, \
             tc.tile_pool(name="scr_ps", bufs=1, space="PSUM") as scr_ps_pool, \
             tc.tile_pool(name="tp_ps", bufs=2, space="PSUM") as tp_ps_pool, \
             tc.tile_pool(name="ctx_ps", bufs=2, space="PSUM") as ctx_ps_pool:
            for b in range(B):
                # scores [24, n]: lhsT = hd strided, rhs = cl_sb strided (CL.T native)
                scr = scr_ps_pool.tile([T, n_steps], F32, tag="scr")
                scn = min(512, n_steps)
                for k in range(KH):
                    for j in range(n_steps // scn):
                        rhs = cl3[:, j * scn:(j + 1) * scn, k, b]
                        nc.tensor.matmul(scr[:, bass.ts(j, scn)], hd_v[:, :, k, b],
                                         rhs, start=(k == 0), stop=(k == KH - 1))
                nmx = att_fix.tile([T, 1], F32, tag="nmx")
                nc.vector.reduce_max(nmx[:], scr[:], axis=mybir.AxisListType.X, negate=True)
                ex = att_fix.tile([T, n_steps], F32, tag="ex")
                sm = att_fix.tile([T, 1], F32, tag="sm")
                nc.scalar.activation(ex[:], scr[:], AF.Exp, bias=nmx[:], accum_out=sm[:])
                rc = att_fix.tile([T, 1], F32, tag="rc")
                nc.vector.reciprocal(rc[:], sm[:])
                cof = att_fix.tile([T, n_steps], BF16, tag="cof")
                nc.vector.tensor_scalar(cof[:], ex[:], rc[:], None, op0=OP.mult)
                # coeff.T [n, 24] via PE transposes
                cT = att_fix.tile([128, nac * T], BF16, tag="cT")
                for j in range(nac):
                    tp = tp_ps_pool.tile([128, 128], BF16, tag="tp")
                    nc.tensor.transpose(tp[:, 0:T], cof[:, bass.ts(j, 128)], id_sb[0:T, 0:T])
                    nc.vector.tensor_copy(cT[:, bass.ts(j, T)], tp[:, 0:T])
                # CL_b n-partitioned tiles via PE transposes
                clb = att_fix.tile([128, nac * KH * 128], BF16, tag="clb")
                for j in range(nac):
                    for k in range(KH):
                        tpc = tp_ps_pool.tile([128, 128], BF16, tag="tp")
                        nc.tensor.transpose(tpc[:], cl3[:, j * 128:(j + 1) * 128, k, b],
                                            id_sb[:, :])
                        nc.vector.tensor_copy(clb[:, bass.ts(j * KH + k, 128)], tpc[:])
                # ctx.T [512, 24]
                ctxp = ctx_ps_pool.tile([128, KH * T], F32, tag="ctxp")
                for k in range(KH):
                    for j in range(nac):
                        nc.tensor.matmul(ctxp[:, bass.ts(k, T)],
                                         clb[:, bass.ts(j * KH + k, 128)],
                                         cT[:, bass.ts(j, T)],
                                         start=(j == 0), stop=(j == nac - 1))
                # out [24, 32]
                ob_ps = scr_ps_pool.tile([T, C], F32, tag="ob_ps")
                for jj in range(8):
                    lr = att_fix.tile([128, T], BF16, tag="lr")
                    src = hd_v[:, :, jj, b] if jj < KH else ctxp[:, bass.ts(jj - KH, T)]
                    nc.scalar.activation(lr[:], src, AF.Lrelu, alpha=0.01)
                    nc.tensor.matmul(ob_ps[:], lr[:], wout_sb[:, bass.ts(jj, C)],
                                     start=(jj == 0), stop=(jj == 7))
                ob = att_fix.tile([T, C], F32, tag="ob")
                nc.vector.tensor_tensor(ob[:], ob_ps[:], bout_sb[:], op=OP.add)
                nc.sync.dma_start(p_out[b], ob[:])
    nc.compile()
    return nc


# m-chunk permutation: new m-position -> original gate chunk index (of 16).
# Original chunks: i=0..3, f=4..7, g=8..11, o=12..15 (PyTorch i,f,g,o order).
# New order: [G0: i0 i1 f0 f1 o0 o1 g0 g1 | G1: i2 i3 f2 f3 o2 o3 g2 g3]
M_PERM = [0, 1, 4, 5, 12, 13, 8, 9,
          2, 3, 6, 7, 14, 15, 10, 11]


def _reorder_cols(Wt):
    """Wt [*, 2048]: permute gate columns into the new m-chunk order."""
    Wn = Wt.reshape(Wt.shape[0], 16, 128)[:, M_PERM, :]
    return np.ascontiguousarray(Wn).reshape(Wt.shape[0], 2048)


def prep_inputs(condition, Wih_enc, Whh_enc, bih_enc, bhh_enc,
                Wih_cell, Whh_cell, bih_cell, bhh_cell, W_out, b_out,
                n_steps=N_STEPS):
    def tile64(Wt):  # [512, 2048] -> [128, 64*128], col block m*4+k
        return np.ascontiguousarray(
            Wt.reshape(KH, 128, M16, 128).transpose(1, 2, 0, 3).reshape(128, 64 * 128)).astype(BF)

    whhT = tile64(_reorder_cols(Whh_enc.T.astype(np.float32)))
    wcellT = tile64(_reorder_cols(Wih_cell.T.astype(np.float32)))
    wih_a = np.zeros((C + 1, G), np.float32)
    wih_a[0:C] = Wih_enc.T
    wih_a[C] = bih_enc + bhh_enc
    wih_a = _reorder_cols(wih_a)
    bias_c = _reorder_cols((bih_cell + bhh_cell).astype(np.float32)[None, :])[0]
    bcell_bc = np.repeat(bias_c.reshape(M16, 128).T[:, :, None], B, axis=2).reshape(128, 256)
    woutT = np.ascontiguousarray(
        W_out.T.reshape(8, 128, C).transpose(1, 0, 2).reshape(128, 8 * C)).astype(BF)
    bout_bc = np.tile(b_out[None, :].astype(np.float32), (T, 1))
    ident = np.eye(128, dtype=np.float32).astype(BF)

    shared = {
        "wihT_a": wih_a.astype(BF), "whhT": whhT, "wcellT": wcellT,
        "bcell_bc": bcell_bc.astype(np.float32), "woutT": woutT,
        "bout_bc": bout_bc, "ident": ident,
    }
    maps = []
    for core in range(NCORES):
        cb = condition[core * B:(core + 1) * B, :n_steps, :]  # [16, n, 32]
        ca = np.ones((C + 1, n_steps * B), np.float32)
        ca[0:C] = cb.transpose(2, 1, 0).reshape(C, n_steps * B)  # col = n*16+b
        m = dict(shared)
        m["cond_aT"] = ca.astype(BF)
        maps.append(m)
    return maps


_NC_CACHE = {}
LAST_RESULT = None


def _ensure_ntff_hook():
    """The agent image's antenv lacks axon_hooks; provide it and register the
    ctypes NTFF profiling hook so trace=True works under axon."""
    import types
    if "antenv.axon_hooks" in sys.modules:
        return
    mod = types.ModuleType("antenv.axon_hooks")
    _h = [None]
    mod.set_axon_ntff_profile_hook = lambda h: _h.__setitem__(0, h)
    mod.get_axon_ntff_profile_hook = lambda: _h[0]
    sys.modules["antenv.axon_hooks"] = mod
    if "/root/.axon_site" not in sys.path:
        sys.path.insert(0, "/root/.axon_site")
    from trn_agent_boot.trn_boot import _ntff_profile_via_ctypes
    mod.set_axon_ntff_profile_hook(_ntff_profile_via_ctypes("/opt/axon/libaxon_pjrt.so"))


def kernel(_trace=False, **inputs):
    global LAST_RESULT
    if _trace:
        try:
            _ensure_ntff_hook()
        except Exception as e:
            print("ntff hook setup failed:", e)
    inputs = {k: np.asarray(v) for k, v in inputs.items()}
    n_steps = N_STEPS
    if n_steps not in _NC_CACHE:
        _NC_CACHE[n_steps] = build_program(n_steps)
    nc = _NC_CACHE[n_steps]
    maps = prep_inputs(**inputs, n_steps=n_steps)
    res = run_bass_kernel_spmd(nc, maps, list(range(NCORES)), trace=_trace)
    LAST_RESULT = res
    out = np.concatenate([np.asarray(res.results[i]["out"], dtype=np.float32)
                          for i in range(NCORES)], axis=0)
    return out
